# revision 3
# baseline (speedup 1.0000x reference)
"""Chamfer loss kernel for 8 Trainium2 NeuronCores.

Problem: ground_truth [4, 8192, 3], reconstruction [4, 8192, 3] (fp32).
  P[b,n,m] = ||x_n||^2 + ||y_m||^2 - 2 x_n.y_m
  loss = (mean(clamp(min_n P)) + mean(clamp(min_m P))) * 1000

Sharding: 8 independent (direction, batch) units -> 1 per core.
  cores 0..3: a = ground_truth[b],  b = reconstruction[b]   (loss_2: min over m)
  cores 4..7: a = reconstruction[b], b = ground_truth[b]    (loss_1: min over n)
Each core returns per-partition partial sums of clamp(min_b dist^2(a_i, b))
over its 8192 a-points; the host sums and combines.

Per-core kernel (v2, fp16 matmul + 2-bank supertile scans):
  - Points are quantized to fp16 on device; all distance terms are then
    computed from the quantized values, so the kernel evaluates EXACT
    squared distances of the quantized cloud (fp16 products are exact in
    fp32). Quantization perturbs each min distance^2 by ~1e-4 relative.
  - min_b(xx + yy - 2xy) = xx + min_b(yy - 2xy): the matmul computes
    P' = yy - 2xy with K=5 fp16 operands:
      lhsT rows [x0, x1, x2, 1, 1]
      rhs  rows [-2y0, -2y1, -2y2, yy_hi, yy_lo]
    yy is split into two fp16 values (hi + lo) to keep fp32-grade
    accuracy through the fp16 operand path. fp16 streams 1 col/cycle on
    the PE (4x the fp32 rate), so no tile_position packing is needed.
  - 64 a-tiles x 8 supertiles of [128, 1024] fp32 (2 PSUM banks each,
    4 rolling buffers = all 8 banks). Per pair of supertiles: ScalarE
    copies the odd one to SBUF ((172+1024)/1.2 = 1.0us), then VectorE
    tensor_tensor_scan(min, min) consumes (psum, sbuf) at ~2 elements/
    lane/cycle ((120+1024)/0.96 = 1.2us), chaining the running min
    through `initial`. DVE is the bottleneck engine at ~4.9us/tile;
    ScalarE ~4.0us; TensorE ~2.1us. All three overlap across tiles.
  - finalize per a-tile: min + xx, clamp at 1e-10 (VectorE tensor_scalar
    so the ScalarE FIFO is never blocked behind a scan); final free-axis
    sum -> [128, 1] per-partition partial output, combined on host.
"""

import sys

if "/opt/trn_rl_repo" not in sys.path:
    sys.path.insert(0, "/opt/trn_rl_repo")

from contextlib import ExitStack

import numpy as np

N = 8192
D = 3
P = 128
NT = N // P  # 64 a-tiles
CH = 512  # transpose/prep group width and matmul free dim
NG = N // CH  # 16 prep groups
STW = 1024  # supertile width (2 PSUM banks)
NPAIR = N // (2 * STW)  # 4 supertile pairs per a-tile

TRACE = False  # set True from test harness to capture an NTFF profile
LAST_RESULTS = None  # BassKernelResults of the most recent run (when traced)

_CACHE = {}


def _build_nc(nt_main=NT, skip_dma=False, variant="v2", reps=1, stw=STW):
    import concourse.bacc as bacc
    import concourse.tile as tile
    from concourse import mybir
    from concourse.masks import make_identity

    f32 = mybir.dt.float32
    f16 = mybir.dt.float16

    nc = bacc.Bacc("TRN2", target_bir_lowering=False, debug=False)

    a_dram = nc.dram_tensor("a_pts", [N, D], f32, kind="ExternalInput")
    b_dram = nc.dram_tensor("b_pts", [N, D], f32, kind="ExternalInput")
    out_dram = nc.dram_tensor("partial", [P, 1], f32, kind="ExternalOutput")

    npair = N // (2 * stw)

    with tile.TileContext(nc) as tc, ExitStack() as ctx:
        consts = ctx.enter_context(tc.tile_pool(name="consts", bufs=1))
        sb = ctx.enter_context(tc.tile_pool(name="sb", bufs=1))
        small = ctx.enter_context(tc.tile_pool(name="small", bufs=2))
        cp_pool = ctx.enter_context(tc.tile_pool(name="cp", bufs=3))
        dst_pool = ctx.enter_context(tc.tile_pool(name="dst", bufs=3))
        prep_ctx = ExitStack()
        prep_ps = prep_ctx.enter_context(
            tc.tile_pool(name="prep_ps", bufs=2, space="PSUM")
        )

        ident = consts.tile([P, P], f32)
        make_identity(nc, ident)

        # natural-layout fp32 staging: [128 points-in-tile, 64 tiles, 3]
        astage = sb.tile([P, NT, D], f32)
        bstage = sb.tile([P, NT, D], f32)
        if skip_dma:
            nc.vector.memset(astage, 0.5)
            nc.vector.memset(bstage, 0.25)
        else:
            nc.sync.dma_start(
                out=astage, in_=a_dram.ap().rearrange("(t p) d -> p t d", p=P)
            )
            nc.sync.dma_start(
                out=bstage, in_=b_dram.ap().rearrange("(t p) d -> p t d", p=P)
            )

        # quantize both clouds to fp16; all downstream math uses the
        # quantized values so distances are exact-in-fp32 of the
        # quantized points
        aq16 = sb.tile([P, NT, D], f16)
        nc.vector.tensor_copy(aq16, astage)
        bq16 = sb.tile([P, NT, D], f16)
        nc.vector.tensor_copy(bq16, bstage)

        # upcast staging of quantized values (transposed on the PE in
        # fp32, downcast again at the PSUM->SBUF copy: exact round trip)
        aqs = sb.tile([P, NT, D], f32)
        nc.vector.tensor_copy(aqs, aq16)
        bq32 = sb.tile([P, NT, D], f32)
        nc.vector.tensor_copy(bq32, bq16)

        # xx per a-point (from quantized coords), natural layout [128, 64]
        sqa = sb.tile([P, NT, D], f32)
        nc.vector.tensor_mul(sqa, aqs, aqs)
        xx = sb.tile([P, NT], f32)
        nc.vector.tensor_reduce(
            out=xx, in_=sqa, axis=mybir.AxisListType.X, op=mybir.AluOpType.add
        )

        # b-side staging [128, 64, 5]: cols 0:3 = -2*y_q, col 3 = yy_hi,
        # col 4 = yy_lo (both stored as upcast-exact fp16 values)
        bqs = sb.tile([P, NT, 5], f32)
        nc.vector.tensor_scalar(
            out=bqs[:, :, 0:D],
            in0=bq32,
            scalar1=-2.0,
            scalar2=None,
            op0=mybir.AluOpType.mult,
        )
        sqb = sb.tile([P, NT, D], f32)
        nc.vector.tensor_mul(sqb, bq32, bq32)
        yy = sb.tile([P, NT, 1], f32)
        nc.vector.tensor_reduce(
            out=yy, in_=sqb, axis=mybir.AxisListType.X, op=mybir.AluOpType.add
        )
        yyh16 = sb.tile([P, NT, 1], f16)
        nc.vector.tensor_copy(yyh16, yy)
        nc.vector.tensor_copy(bqs[:, :, 3:4], yyh16)  # upcast yy_hi
        resid = sb.tile([P, NT, 1], f32)
        nc.vector.tensor_tensor(
            out=resid, in0=yy, in1=bqs[:, :, 3:4], op=mybir.AluOpType.subtract
        )
        yyl16 = sb.tile([P, NT, 1], f16)
        nc.vector.tensor_copy(yyl16, resid)
        nc.vector.tensor_copy(bqs[:, :, 4:5], yyl16)  # upcast yy_lo

        # K-major fp16 operands: LHS rows [x0,x1,x2,1,1], RHS rows
        # [-2y0,-2y1,-2y2,yy_hi,yy_lo]
        LHS = sb.tile([5, N], f16)
        RHS = sb.tile([5, N], f16)
        # rows 3:5 stay 1.0 (the ones rows); rows 0:3 are overwritten by the
        # per-group transposed-coordinate copies below
        nc.vector.memset(LHS, 1.0)
        for g in range(NG):
            tpa = prep_ps.tile([D, CH], f32, tag="tpa")
            for c in range(4):
                t = 4 * g + c
                nc.tensor.transpose(tpa[:, c * P : (c + 1) * P], aqs[:, t, :], ident)
            nc.scalar.copy(LHS[0:D, g * CH : (g + 1) * CH], tpa)  # downcast
            tpb = prep_ps.tile([5, CH], f32, tag="tpb")
            for c in range(4):
                t = 4 * g + c
                nc.tensor.transpose(tpb[:, c * P : (c + 1) * P], bqs[:, t, :], ident)
            nc.scalar.copy(RHS[:, g * CH : (g + 1) * CH], tpb)  # downcast

        prep_ctx.close()
        psum_bufs = (2 * 1024) // stw * 2  # 4 bufs at stw=1024, 8 at stw=512
        main_ps = ctx.enter_context(
            tc.tile_pool(name="main_ps", bufs=psum_bufs, space="PSUM")
        )

        res = sb.tile([P, NT], f32)
        if nt_main < NT:
            nc.vector.memset(res, 0.0)

        rep_ctx = ExitStack()
        if reps > 1:  # timing amplification: re-execute the main loop
            rep_ctx.enter_context(tc.For_i(0, reps, 1))

        mm_per_st = stw // CH
        for t in range(nt_main):
            lhs_t = LHS[:, t * P : (t + 1) * P]
            prev_init = None  # AP of the running min ([P,1]) or None
            for k in range(npair):
                base = k * 2 * stw
                pb0 = main_ps.tile([P, stw], f32, tag="st")
                pb1 = main_ps.tile([P, stw], f32, tag="st")
                for h in range(mm_per_st):
                    nc.tensor.matmul(
                        pb0[:, h * CH : (h + 1) * CH],
                        lhs_t,
                        RHS[:, base + h * CH : base + (h + 1) * CH],
                        start=True,
                        stop=True,
                    )
                for h in range(mm_per_st):
                    nc.tensor.matmul(
                        pb1[:, h * CH : (h + 1) * CH],
                        lhs_t,
                        RHS[:, base + stw + h * CH : base + stw + (h + 1) * CH],
                        start=True,
                        stop=True,
                    )
                if variant == "mmonly":
                    continue
                cp = cp_pool.tile([P, stw], f32, tag="cp")
                nc.scalar.copy(cp, pb1)
                dst = dst_pool.tile([P, stw], f32, tag="dst")
                nc.vector.tensor_tensor_scan(
                    out=dst,
                    data0=pb0,
                    initial=(1.0e30 if prev_init is None else prev_init),
                    data1=cp,
                    op0=mybir.AluOpType.min,
                    op1=mybir.AluOpType.min,
                )
                prev_init = dst[:, stw - 1 : stw]
            if variant == "mmonly":
                continue
            # res[:, t] = max(min + xx[:, t], 1e-10)
            nc.vector.tensor_scalar(
                out=res[:, t : t + 1],
                in0=prev_init,
                scalar1=xx[:, t : t + 1],
                scalar2=1e-10,
                op0=mybir.AluOpType.add,
                op1=mybir.AluOpType.max,
            )

        rep_ctx.close()
        if variant == "mmonly":
            nc.vector.memset(res, 7.0)

        res1 = small.tile([P, 1], f32)
        nc.vector.tensor_reduce(
            out=res1, in_=res, axis=mybir.AxisListType.X, op=mybir.AluOpType.add
        )
        nc.sync.dma_start(out=out_dram.ap(), in_=res1)

    nc.compile()
    return nc


def _get_nc(**kw):
    key = tuple(sorted(kw.items())) or "nc"
    if key not in _CACHE:
        _CACHE[key] = _build_nc(**kw)
    return _CACHE[key]


def _run(nc, gt, rc, B):
    from concourse.bass_utils import run_bass_kernel_spmd

    in_maps = []
    for b in range(B):  # cores 0..3: min over reconstruction for each gt point
        in_maps.append({"a_pts": gt[b], "b_pts": rc[b]})
    for b in range(B):  # cores 4..7: min over gt for each reconstruction point
        in_maps.append({"a_pts": rc[b], "b_pts": gt[b]})

    try:
        results = run_bass_kernel_spmd(
            nc, in_maps, core_ids=list(range(2 * B)), trace=TRACE
        )
    except Exception:
        # transient NRT_EXEC_UNIT_UNRECOVERABLE has been observed after
        # heavy preceding runs; one retry recovers
        results = run_bass_kernel_spmd(
            nc, in_maps, core_ids=list(range(2 * B)), trace=TRACE
        )
    return results


def kernel(ground_truth: np.ndarray, reconstruction: np.ndarray) -> np.ndarray:
    global LAST_RESULTS

    gt = np.ascontiguousarray(ground_truth, dtype=np.float32)
    rc = np.ascontiguousarray(reconstruction, dtype=np.float32)
    B = gt.shape[0]
    assert gt.shape == (B, N, D) and rc.shape == (B, N, D)

    nc = _get_nc()
    results = _run(nc, gt, rc, B)
    LAST_RESULTS = results

    partials = np.array(
        [float(np.sum(r["partial"].astype(np.float64))) for r in results.results]
    )
    loss_2 = partials[:B].sum() / (B * N)
    loss_1 = partials[B:].sum() / (B * N)
    total = (loss_1 + loss_2) * 1000.0
    return np.asarray(total, dtype=np.float32)
